# revision 21
# baseline (speedup 1.0000x reference)
"""Trainium2 Bass kernel for the DIST loss (inter spearman-variant + intra
pearson).

Contract: kernel(z_s, z_t) -> scalar np.float32 () matching
reference.reference.

v3 strategy (8 cores, batch-sharded 512 rows/core):
  - u = exp(z - 2) f16, one ACT pass per [P,4000] block, accum -> row sums.
  - Rank counts r_c = #{k: u_k < u_c}, c = 0..9, per tensor:
      * n_A units on ACT: Sign(bias-u) + accum (1x, 16.7us/unit)
      * n_D units on DVE: tensor_scalar is_lt 4x (5.3us) + in-place
        fold-add tree at 2x (L1..L4) + short 1x reduce  (~15.8us/unit)
  - argmax via two tournaments (T-fold strided combs -> 250, N-fold 64
    blocks of 250 -> 64), both as full-width 2x TT max chains on DVE.
  - Intra stats via PE matmuls (stationary = data chunk [128,125],
    moving = per-strip scaled weight column), squares/cross on GPSIMD.
  - bf16 ReduceScatter of the 5x16000 stats; per-rank pearson shard;
    tiny f32 AllReduce for the scalars.
"""

import sys

import numpy as np

sys.path.insert(0, "/opt/trn_rl_repo")

# ---------------------------------------------------------------- constants
B_FULL = 4096
C = 16000
N_CORES = 8
RPC = B_FULL // N_CORES  # rows per core = 512
P = 128
NSTRIP = RPC // P        # 4
ZB = 2000                # dma/exp column block
NZB = C // ZB            # 4
HB = 8000                # half-row block for DVE counts / ACT scratch
MP = 125                 # stationary chunk classes
NCH = C // MP            # 128 chunks
NSTATS = 5
EXP_BIAS = 2.0
RANK_CLAMP = 10
EPS = 1e-8
A_SC = 2.0 ** 8          # scale on 1/S weights (f16 headroom)
B_SC = 2.0 ** 16

# engine per (tensor, class): 'a' = ACT sign, 'v' = DVE cmp+fold.
CMP_ENGINE = {}
for _t in range(2):
    for _c in range(RANK_CLAMP):
        CMP_ENGINE[(_t, _c)] = "a" if _c < (6 - _t) else "v"


def build_program(rpc=RPC, c=C, n_cores=N_CORES):
    import concourse.bass as bass
    import concourse.mybir as mybir
    import concourse.tile as tile
    from concourse import bacc
    from concourse.alu_op_type import AluOpType as OP

    f32 = mybir.dt.float32
    f16 = mybir.dt.float16
    bf16 = mybir.dt.bfloat16
    u32 = mybir.dt.uint32
    ACT = mybir.ActivationFunctionType
    AX = mybir.AxisListType

    nstrip = rpc // P
    inv_n = 1.0 / (c - 1)

    nc = bacc.Bacc(None, target_bir_lowering=False, debug=False,
                   num_devices=n_cores)

    z_s = nc.declare_dram_parameter("z_s", [rpc, c], f32, isOutput=False)
    z_t = nc.declare_dram_parameter("z_t", [rpc, c], f32, isOutput=False)
    out = nc.declare_dram_parameter("out", [1, 1], f32, isOutput=True)

    def bcast(ap, dims):
        return bass.AP(tensor=ap.tensor, offset=ap.offset,
                       ap=[ap.ap[0]] + dims)

    from contextlib import ExitStack
    with tile.TileContext(nc) as tc, ExitStack() as ctx:
        zpool = ctx.enter_context(tc.tile_pool(name="zpool", bufs=2))
        upool = ctx.enter_context(tc.tile_pool(name="upool", bufs=2))
        scrp = ctx.enter_context(tc.tile_pool(name="scrp", bufs=1))
        sqpool = ctx.enter_context(tc.tile_pool(name="sqpool", bufs=2))
        small = ctx.enter_context(tc.tile_pool(name="small", bufs=1))
        stiny = ctx.enter_context(tc.tile_pool(name="stiny", bufs=2))
        psum = ctx.enter_context(tc.tile_pool(name="psum", bufs=2,
                                              space="PSUM"))
        dram = ctx.enter_context(tc.tile_pool(name="dram", bufs=1,
                                              space="DRAM"))

        # ---------------- persistent small tiles ----------------
        cnt = small.tile([P, nstrip, 2, RANK_CLAMP, 4], f32, tag="cnt")
        nc.vector.memset(cnt[:], 0.0)
        ssum = small.tile([P, nstrip, 2, NZB], f32, tag="ssum")
        thetas = small.tile([P, nstrip, 2, RANK_CLAMP], f32, tag="thetas")
        bt = small.tile([P, nstrip, 2, 2], f32, tag="bt")  # [b*, t*]
        stats_acc = small.tile([MP, NSTATS, NCH], f32, tag="stats_acc")
        nc.vector.memset(stats_acc[:], 0.0)
        stats_acc2 = small.tile([MP, NSTATS, NCH], f32, tag="stats_acc2")
        nc.vector.memset(stats_acc2[:], 0.0)
        nbias = small.tile([P, 1], f32, tag="nbias")
        nc.vector.memset(nbias[:], -EXP_BIAS)
        zbias = small.tile([P, 1], f32, tag="zbias")
        nc.vector.memset(zbias[:], 0.0)
        ones_col = small.tile([P, 1], f32, tag="ones_col")
        nc.vector.memset(ones_col[:], 1.0)

        cc_in = dram.tile([P, NSTATS * NCH], bf16, tag="cc_in")
        cc_out = dram.tile([16, NSTATS * NCH], bf16, tag="cc_out")
        cc_in2 = dram.tile([P, NSTATS * NCH], bf16, tag="cc_in2")
        cc_out2 = dram.tile([16, NSTATS * NCH], bf16, tag="cc_out2")
        ar_in = dram.tile([1, 4], f32, tag="ar_in")
        ar_out = dram.tile([1, 4], f32, tag="ar_out")

        # zero pad rows 125..127 of cc_in once
        zpad = small.tile([3, NSTATS * NCH], bf16, tag="zpad")
        nc.vector.memset(zpad[:], 0.0)
        nc.sync.dma_start(out=cc_in[MP:P, :], in_=zpad[:])
        nc.sync.dma_start(out=cc_in2[MP:P, :], in_=zpad[:])

        # scratch (single-buffered; consumers are same-engine in-order)
        scr_a = scrp.tile([P, HB], f16, tag="scr_a")     # ACT sign output
        vscr = scrp.tile([P, HB], f16, tag="vscr")       # DVE bitmap + folds


        def rsqrt_dve(dst, srcf, scr_f, scr_f2, y0):
            """dst = 1/sqrt(srcf), DVE only (no ACT table switch).
            Newton from constant seed y0 with clamped factor: globally
            convergent from above for any positive srcf."""
            y = scr_f
            t1 = scr_f2
            nc.vector.memset(y, y0)
            for _ in range(6):
                nc.vector.tensor_tensor(t1, y, y, OP.mult)
                nc.vector.tensor_tensor(t1, t1, srcf, OP.mult)
                nc.vector.tensor_scalar(t1, t1, -0.5, 1.5,
                                        OP.mult, OP.add)
                nc.vector.tensor_scalar_max(t1, t1, 0.25)
                nc.vector.tensor_tensor(y, y, t1, OP.mult)
            nc.vector.tensor_copy(dst, y)

        # =================== main strip loop ===================
        for s in range(nstrip):
            us = []
            # ---- DMA + exp (per z-block), u f16 [P, 16000] per tensor
            for t, zp in ((0, z_s), (1, z_t)):
                uq = upool.tile([P, c], f16, tag=f"u{t}")
                us.append(uq)
                for h in range(NZB):
                    zb = zpool.tile([P, ZB], f32, tag="zb")
                    col0 = h * ZB
                    nc.sync.dma_start(
                        out=zb[:],
                        in_=zp[s * P:(s + 1) * P, col0:col0 + ZB])
                    nc.scalar.activation(
                        uq[:, col0:col0 + ZB], zb[:], ACT.Exp,
                        bias=nbias[:], scale=1.0,
                        accum_out=ssum[:, s, t, h:h + 1])

            # ---- thetas from cols 0..9 (on ACT: keeps the ACT stream
            # self-contained so strip s+1 counts never wait on DVE)
            for t in range(2):
                nc.scalar.activation(thetas[:, s, t, :], us[t][:, 0:10],
                                     ACT.Copy, bias=0.0, scale=1.0)

            # ---- weights from row sums
            sS = stiny.tile([P, 2], f32, tag="sS")
            nc.vector.reduce_sum(sS[:, 0:1], ssum[:, s, 0, :], axis=AX.X)
            nc.vector.reduce_sum(sS[:, 1:2], ssum[:, s, 1, :], axis=AX.X)
            rr = stiny.tile([P, 2], f32, tag="rr")
            nc.vector.reciprocal(rr[:], sS[:])
            wf = stiny.tile([P, NSTATS], f32, tag="wf")
            nc.vector.tensor_scalar(wf[:, 0:1], rr[:, 0:1], A_SC, None,
                                    OP.mult)
            nc.vector.tensor_scalar(wf[:, 2:3], rr[:, 1:2], A_SC, None,
                                    OP.mult)
            r2 = stiny.tile([P, 3], f32, tag="r2")
            nc.vector.tensor_tensor(r2[:, 0:1], rr[:, 0:1], rr[:, 0:1],
                                    OP.mult)
            nc.vector.tensor_tensor(r2[:, 1:2], rr[:, 1:2], rr[:, 1:2],
                                    OP.mult)
            nc.vector.tensor_tensor(r2[:, 2:3], rr[:, 0:1], rr[:, 1:2],
                                    OP.mult)
            nc.vector.tensor_scalar(wf[:, 1:2], r2[:, 0:1], B_SC, None,
                                    OP.mult)
            nc.vector.tensor_scalar(wf[:, 3:4], r2[:, 1:2], B_SC, None,
                                    OP.mult)
            nc.vector.tensor_scalar(wf[:, 4:5], r2[:, 2:3], B_SC, None,
                                    OP.mult)
            wcol = stiny.tile([P, NSTATS], f16, tag="wcol")
            nc.vector.tensor_copy(wcol[:], wf[:])

            # ---- A-stats (u-stationary) early: PE runs these while
            # counts are in flight, before the GPS squares convoy
            stats_ps = psum.tile([MP, NSTATS, NCH], f32, tag="stats_ps")
            for kk in range(NCH):
                ksl = slice(kk * MP, (kk + 1) * MP)
                nc.tensor.matmul(stats_ps[0:MP, 0, kk:kk + 1],
                                 us[0][:, ksl], wcol[:, 0:1],
                                 start=True, stop=True)
                nc.tensor.matmul(stats_ps[0:MP, 2, kk:kk + 1],
                                 us[1][:, ksl], wcol[:, 2:3],
                                 start=True, stop=True)

            # ---- counts (20 units)
            for cc_ in range(RANK_CLAMP):
                for t in range(2):
                    th = thetas[:, s, t, cc_:cc_ + 1]
                    uq = us[t]
                    if CMP_ENGINE[(t, cc_)] == "a":
                        for g in range(2):
                            nc.scalar.activation(
                                scr_a[:], uq[:, g * HB:(g + 1) * HB],
                                ACT.Sign, bias=th, scale=-1.0,
                                accum_out=cnt[:, s, t, cc_, g:g + 1])
                    else:
                        for hh in range(2):
                            nc.vector.tensor_scalar(
                                vscr[:], uq[:, hh * HB:(hh + 1) * HB],
                                th, None, OP.is_lt)
                            # fold-add tree: 8000 -> 500 (in-place, f16
                            # partial sums stay exact: max value 16 < 2048)
                            w = HB // 2
                            while w >= 500:
                                nc.vector.tensor_tensor(
                                    vscr[:, 0:w], vscr[:, 0:w],
                                    vscr[:, w:2 * w], OP.add)
                                w //= 2
                            nc.vector.reduce_sum(
                                cnt[:, s, t, cc_, hh:hh + 1],
                                vscr[:, 0:500], axis=AX.X)

            # ---- argmax tournaments per tensor (full width)
            for t in range(2):
                uq = us[t]
                # T-fold: contiguous halving 16000 -> 250 into vscr
                T_glob = stiny.tile([P, 250], f16, tag="T_glob")
                nc.vector.tensor_tensor(vscr[:], uq[:, 0:8000],
                                        uq[:, 8000:16000], OP.max)
                w = HB // 2
                while w >= 250:
                    nc.vector.tensor_tensor(vscr[:, 0:w], vscr[:, 0:w],
                                            vscr[:, w:2 * w], OP.max)
                    w //= 2
                nc.vector.tensor_copy(T_glob[:], vscr[:, 0:250])
                # N-fold: 64 blocks of 250 -> [P, 64]
                M_N = stiny.tile([P, 64], f16, tag="M_N")
                vN = uq[:].rearrange("p (b i) -> p b i", b=64)
                tN = vscr[:].rearrange("p (b i) -> p b i", b=64)
                nc.vector.tensor_tensor(tN[:, :, 0:125], vN[:, :, 0:125],
                                        vN[:, :, 125:250], OP.max)
                curn = 125
                while curn > 1:
                    half = curn // 2
                    rem = curn - half  # fold upper half onto lower
                    nc.vector.tensor_tensor(
                        tN[:, :, 0:half], tN[:, :, 0:half],
                        tN[:, :, rem:curn], OP.max)
                    curn = rem
                nc.vector.tensor_copy(M_N[:], tN[:, :, 0])
                # finalize: j = 250*argmax(M_N) + argmax(T_glob)
                m8 = stiny.tile([P, 8], f16, tag="m8")
                i8 = stiny.tile([P, 8], u32, tag="i8")
                nc.vector.max(m8[:], M_N[:])
                m8b = bcast(m8[:, 0:1], [[0, 8]])
                nc.vector.max_index(i8[:], m8b, M_N[:])
                nc.vector.tensor_copy(bt[:, s, t, 0:1], i8[:, 0:1])
                nc.vector.max_index(i8[:], m8b, T_glob[:])
                nc.vector.tensor_copy(bt[:, s, t, 1:2], i8[:, 0:1])

            # ---- squares / cross on GPSIMD, consumed by PE
            SQB = 1000
            for q in range(c // SQB):
                qsl = slice(q * SQB, (q + 1) * SQB)
                sq_s = sqpool.tile([P, SQB], bf16, tag="sq_s")
                sq_t = sqpool.tile([P, SQB], bf16, tag="sq_t")
                xst = sqpool.tile([P, SQB], bf16, tag="xst")
                nc.gpsimd.tensor_tensor(sq_s[:], us[0][:, qsl], us[0][:, qsl],
                                        OP.mult)
                nc.gpsimd.tensor_tensor(sq_t[:], us[1][:, qsl], us[1][:, qsl],
                                        OP.mult)
                nc.gpsimd.tensor_tensor(xst[:], us[0][:, qsl], us[1][:, qsl],
                                        OP.mult)
                for k in range(SQB // MP):  # 16 chunks / block
                    kk = q * (SQB // MP) + k
                    ksl = slice(q * SQB + k * MP, q * SQB + (k + 1) * MP)
                    ksl_l = slice(k * MP, (k + 1) * MP)
                    lhss = ((sq_s[:, ksl_l], 1), (sq_t[:, ksl_l], 3),
                            (xst[:, ksl_l], 4))
                    for lhsT, si in lhss:
                        nc.tensor.matmul(
                            stats_ps[0:MP, si, kk:kk + 1],
                            lhsT, wcol[:, si:si + 1],
                            start=True, stop=True)
            sacc = stats_acc if s < 2 else stats_acc2
            nc.vector.tensor_tensor(sacc[:], sacc[:],
                                    stats_ps[0:MP, :, :], OP.add)
            if s == 3:
                stats_bf2 = small.tile([MP, NSTATS * NCH], bf16,
                                       tag="stats_bf2")
                nc.vector.tensor_copy(
                    stats_bf2[:],
                    stats_acc2[:].rearrange("p a b -> p (a b)"))
                nc.sync.dma_start(out=cc_in2[0:MP, :], in_=stats_bf2[:])
                nc.gpsimd.collective_compute(
                    "ReduceScatter", OP.add,
                    replica_groups=[list(range(n_cores))],
                    ins=[cc_in2[:].opt()], outs=[cc_out2[:].opt()])
            if s == 1:
                # first-half stats: start the ReduceScatter now so it
                # overlaps strips 2-3
                stats_bf1 = small.tile([MP, NSTATS * NCH], bf16,
                                       tag="stats_bf1")
                nc.vector.tensor_copy(
                    stats_bf1[:], stats_acc[:].rearrange("p a b -> p (a b)"))
                nc.sync.dma_start(out=cc_in[0:MP, :], in_=stats_bf1[:])
                nc.gpsimd.collective_compute(
                    "ReduceScatter", OP.add,
                    replica_groups=[list(range(n_cores))],
                    ins=[cc_in[:].opt()], outs=[cc_out[:].opt()])

        # ================= inter-term combine =================
        # js/jt = 250*b + t
        js = small.tile([P, nstrip], f32, tag="js")
        jt = small.tile([P, nstrip], f32, tag="jt")
        for t, jx in ((0, js), (1, jt)):
            nc.vector.tensor_scalar(jx[:], bt[:, :, t, 0], 250.0, None,
                                    OP.mult)
            nc.vector.tensor_tensor(jx[:], jx[:], bt[:, :, t, 1], OP.add)

        # counts: reduce over halves
        cr = small.tile([P, nstrip, 2, RANK_CLAMP, 1], f32, tag="cr")
        nc.vector.reduce_sum(cr[:], cnt[:], axis=AX.X)
        # ACT units: sign-sum acc = 2*r - (c-1) over non-self elements ->
        # r = 0.5*acc + 0.5*(c-1)
        for t in range(2):
            for cc_ in range(RANK_CLAMP):
                if CMP_ENGINE[(t, cc_)] == "a":
                    v = cr[:, :, t, cc_, 0]
                    nc.vector.tensor_scalar(v, v, float(c - 1), 0.5,
                                            OP.add, OP.mult)

        crs2 = cr[:, :, 0, :, 0]  # [P, strip, 10]
        crt2 = cr[:, :, 1, :, 0]
        js_b = bcast(js[:], [[1, nstrip], [0, RANK_CLAMP]])
        jt_b = bcast(jt[:], [[1, nstrip], [0, RANK_CLAMP]])
        gt_s = small.tile([P, nstrip, 10], f32, tag="gt_s")
        gt_t = small.tile([P, nstrip, 10], f32, tag="gt_t")
        kp_s = small.tile([P, nstrip, 10], f32, tag="kp_s")
        kp_t = small.tile([P, nstrip, 10], f32, tag="kp_t")
        p_s = small.tile([P, nstrip, 10], f32, tag="p_s")
        p_t = small.tile([P, nstrip, 10], f32, tag="p_t")
        for crx, jb, gt, kp, px, sent in (
                (crs2, js_b, gt_s, kp_s, p_s, 5.0),
                (crt2, jt_b, gt_t, kp_t, p_t, 7.0)):
            nc.vector.tensor_tensor(gt[:], crx, jb, OP.is_gt)
            nc.vector.tensor_tensor(kp[:], crx, jb, OP.not_equal)
            nc.vector.tensor_tensor(px[:], crx, gt[:], OP.subtract)
            nc.vector.tensor_scalar_add(px[:], px[:], sent)
            nc.vector.tensor_tensor(px[:], px[:], kp[:], OP.mult)
            nc.vector.tensor_scalar_add(px[:], px[:], -sent)

        wa = small.tile([P, 10], f32, tag="wa")
        for cc_ in range(RANK_CLAMP):
            nc.vector.memset(wa[:, cc_:cc_ + 1], float(cc_ - RANK_CLAMP))
        wa_b = bcast(wa[:], [[0, nstrip], [1, 10]])
        kw_s = small.tile([P, nstrip, 10], f32, tag="kw_s")
        kw_t = small.tile([P, nstrip, 10], f32, tag="kw_t")
        nc.vector.tensor_tensor(kw_s[:], kp_s[:], wa_b, OP.mult)
        nc.vector.tensor_tensor(kw_t[:], kp_t[:], wa_b, OP.mult)
        s1_s = small.tile([P, nstrip, 1], f32, tag="s1_s")
        s1_t = small.tile([P, nstrip, 1], f32, tag="s1_t")
        nc.vector.reduce_sum(s1_s[:], kw_s[:], axis=AX.X)
        nc.vector.reduce_sum(s1_t[:], kw_t[:], axis=AX.X)
        k2_s = small.tile([P, nstrip, 10], f32, tag="k2_s")
        k2_t = small.tile([P, nstrip, 10], f32, tag="k2_t")
        nc.vector.tensor_tensor(k2_s[:], kw_s[:], wa_b, OP.mult)
        nc.vector.tensor_tensor(k2_t[:], kw_t[:], wa_b, OP.mult)
        s2_s = small.tile([P, nstrip, 1], f32, tag="s2_s")
        s2_t = small.tile([P, nstrip, 1], f32, tag="s2_t")
        nc.vector.reduce_sum(s2_s[:], k2_s[:], axis=AX.X)
        nc.vector.reduce_sum(s2_t[:], k2_t[:], axis=AX.X)

        w100 = small.tile([P, 100], f32, tag="w100")
        nc.vector.tensor_tensor(
            w100[:],
            bcast(wa[:], [[1, 10], [0, 10]]),
            bcast(wa[:], [[0, 10], [1, 10]]), OP.mult)
        eq = small.tile([P, nstrip, 10, 10], f32, tag="eq")
        nc.vector.tensor_tensor(
            eq[:],
            bcast(p_s[:], [[10, nstrip], [1, 10], [0, 10]]),
            bcast(p_t[:], [[10, nstrip], [0, 10], [1, 10]]), OP.is_equal)
        nc.vector.tensor_tensor(
            eq[:], eq[:],
            bcast(w100[:], [[0, nstrip], [10, 10], [1, 10]]), OP.mult)
        xterm = small.tile([P, nstrip, 1, 1], f32, tag="xterm")
        nc.vector.reduce_sum(xterm[:], eq[:], axis=AX.XY)

        x2 = xterm[:, :, 0, 0]
        num = small.tile([P, nstrip], f32, tag="num")
        nc.vector.tensor_tensor(num[:], s1_s[:, :, 0], s1_t[:, :, 0], OP.mult)
        nc.vector.scalar_tensor_tensor(
            num[:], num[:], -inv_n, x2, OP.mult, OP.add)
        var_s = small.tile([P, nstrip], f32, tag="var_s")
        var_t = small.tile([P, nstrip], f32, tag="var_t")
        for s1x, s2x, varx in ((s1_s, s2_s, var_s), (s1_t, s2_t, var_t)):
            nc.vector.tensor_tensor(varx[:], s1x[:, :, 0], s1x[:, :, 0],
                                    OP.mult)
            nc.vector.scalar_tensor_tensor(
                varx[:], varx[:], -inv_n, s2x[:, :, 0], OP.mult, OP.add)
        den = small.tile([P, nstrip], f32, tag="den")
        nc.vector.tensor_tensor(den[:], var_s[:], var_t[:], OP.mult)
        rsqrt_dve(var_s[:], den[:], var_t[:], gt_s[:, :, 0], 3.0e-3)
        rho = small.tile([P, nstrip], f32, tag="rho")
        nc.vector.tensor_tensor(rho[:], num[:], var_s[:], OP.mult)
        eqj = small.tile([P, nstrip], f32, tag="eqj")
        nc.vector.tensor_tensor(eqj[:], js[:], jt[:], OP.is_equal)

        packed = small.tile([P, 2], f32, tag="packed")
        nc.vector.reduce_sum(packed[:, 0:1], rho[:], axis=AX.X)
        nc.vector.reduce_sum(packed[:, 1:2], eqj[:], axis=AX.X)
        inter_ps = psum.tile([1, 2], f32, tag="inter_ps")
        nc.tensor.matmul(inter_ps[:], ones_col[:], packed[:],
                         start=True, stop=True)
        inter_sb = small.tile([1, 2], f32, tag="inter_sb")
        nc.vector.tensor_copy(inter_sb[:], inter_ps[:])

        # ================= stats collective (bf16) =================
        # (RS2 already emitted at end of strip 3; RS1 after strip 1)
        # per-rank pearson shard: [16, 5, 128] = sum of both halves
        shb = small.tile([16, NSTATS, NCH], bf16, tag="shb")
        nc.sync.dma_start(out=shb[:].rearrange("p a b -> p (a b)"),
                          in_=cc_out[:])
        shb2 = small.tile([16, NSTATS, NCH], bf16, tag="shb2")
        nc.sync.dma_start(out=shb2[:].rearrange("p a b -> p (a b)"),
                          in_=cc_out2[:])
        sh = small.tile([16, NSTATS, NCH], f32, tag="sh")
        nc.vector.tensor_tensor(sh[:], shb[:], shb2[:], OP.add)
        a_s, b_s, a_t, b_t, e_st = (sh[:, i, :] for i in range(NSTATS))
        inv_b = 1.0 / (rpc * n_cores)
        num2 = small.tile([16, NCH], f32, tag="num2")
        nc.vector.tensor_tensor(num2[:], a_s, a_t, OP.mult)
        nc.vector.scalar_tensor_tensor(
            num2[:], num2[:], -inv_b, e_st, OP.mult, OP.add)
        va = small.tile([16, NCH], f32, tag="va")
        vb = small.tile([16, NCH], f32, tag="vb")
        for ax, bx, vx in ((a_s, b_s, va), (a_t, b_t, vb)):
            nc.vector.tensor_tensor(vx[:], ax, ax, OP.mult)
            nc.vector.scalar_tensor_tensor(
                vx[:], vx[:], -inv_b, bx, OP.mult, OP.add)
        den2 = small.tile([16, NCH], f32, tag="den2")
        nc.vector.tensor_tensor(den2[:], va[:], vb[:], OP.mult)
        rsqrt_dve(va[:], den2[:], vb[:], sh[:, 1, :], 0.55)
        nc.vector.tensor_tensor(num2[:], num2[:], va[:], OP.mult)
        rho_cls = small.tile([16, 1], f32, tag="rho_cls")
        nc.vector.reduce_sum(rho_cls[:], num2[:], axis=AX.X)
        intra_ps = psum.tile([1, 1], f32, tag="intra_ps")
        nc.tensor.matmul(intra_ps[:], ones_col[0:16, :], rho_cls[:],
                         start=True, stop=True)

        # tiny AllReduce: [intra_shard, rho_sum, eq_sum, 0]
        sc4 = small.tile([1, 4], f32, tag="sc4")
        nc.vector.memset(sc4[:], 0.0)
        nc.vector.tensor_copy(sc4[:, 0:1], intra_ps[:])
        nc.vector.tensor_copy(sc4[:, 1:3], inter_sb[:])
        nc.sync.dma_start(out=ar_in[:], in_=sc4[:])
        nc.gpsimd.collective_compute(
            "AllReduce", OP.add,
            replica_groups=[list(range(n_cores))],
            ins=[ar_in[:].opt()], outs=[ar_out[:].opt()])
        scf = small.tile([1, 4], f32, tag="scf")
        nc.sync.dma_start(out=scf[:], in_=ar_out[:])

        # fin = 2 - (rho_sum + eq_sum)/B - intra_sum/C
        fin = small.tile([1, 1], f32, tag="fin")
        nc.vector.tensor_tensor(fin[:], scf[:, 1:2], scf[:, 2:3], OP.add)
        nc.vector.tensor_scalar_mul(fin[:], fin[:], -inv_b)
        nc.vector.scalar_tensor_tensor(
            fin[:], scf[:, 0:1], -1.0 / c, fin[:], OP.mult, OP.add)
        nc.vector.tensor_scalar_add(fin[:], fin[:], 2.0)
        nc.sync.dma_start(out=out[:], in_=fin[:])

    nc.finalize()
    return nc


_CACHED = {}


def _get_program():
    if "nc" not in _CACHED:
        _CACHED["nc"] = build_program()
    return _CACHED["nc"]


def kernel(z_s: np.ndarray, z_t: np.ndarray) -> np.ndarray:
    from concourse.bass_utils import run_bass_kernel_spmd

    nc = _get_program()
    in_maps = []
    for i in range(N_CORES):
        sl = slice(i * RPC, (i + 1) * RPC)
        in_maps.append({
            "z_s": np.ascontiguousarray(z_s[sl], dtype=np.float32),
            "z_t": np.ascontiguousarray(z_t[sl], dtype=np.float32),
        })
    res = run_bass_kernel_spmd(nc, in_maps, core_ids=list(range(N_CORES)))
    val = np.asarray(res.results[0]["out"], dtype=np.float32).reshape(())
    return val


# revision 23
# speedup vs baseline: 1.0570x; 1.0570x over previous
"""Trainium2 Bass kernel for the DIST loss (inter spearman-variant + intra
pearson).

Contract: kernel(z_s, z_t) -> scalar np.float32 () matching
reference.reference.

v3 strategy (8 cores, batch-sharded 512 rows/core):
  - u = exp(z - 2) f16, one ACT pass per [P,4000] block, accum -> row sums.
  - Rank counts r_c = #{k: u_k < u_c}, c = 0..9, per tensor:
      * n_A units on ACT: Sign(bias-u) + accum (1x, 16.7us/unit)
      * n_D units on DVE: tensor_scalar is_lt 4x (5.3us) + in-place
        fold-add tree at 2x (L1..L4) + short 1x reduce  (~15.8us/unit)
  - argmax via two tournaments (T-fold strided combs -> 250, N-fold 64
    blocks of 250 -> 64), both as full-width 2x TT max chains on DVE.
  - Intra stats via PE matmuls (stationary = data chunk [128,125],
    moving = per-strip scaled weight column), squares/cross on GPSIMD.
  - bf16 ReduceScatter of the 5x16000 stats; per-rank pearson shard;
    tiny f32 AllReduce for the scalars.
"""

import sys

import numpy as np

sys.path.insert(0, "/opt/trn_rl_repo")

# ---------------------------------------------------------------- constants
B_FULL = 4096
C = 16000
N_CORES = 8
RPC = B_FULL // N_CORES  # rows per core = 512
P = 128
NSTRIP = RPC // P        # 4
ZB = 2000                # dma/exp column block
NZB = C // ZB            # 4
HB = 8000                # half-row block for DVE counts / ACT scratch
MP = 125                 # stationary chunk classes
NCH = C // MP            # 128 chunks
NSTATS = 5
EXP_BIAS = 2.0
RANK_CLAMP = 10
EPS = 1e-8
A_SC = 2.0 ** 8          # scale on 1/S weights (f16 headroom)
B_SC = 2.0 ** 16

# engine per (tensor, class): 'a' = ACT sign, 'v' = DVE cmp+fold.
CMP_ENGINE = {}
for _t in range(2):
    for _c in range(RANK_CLAMP):
        CMP_ENGINE[(_t, _c)] = "a" if _c < 6 else "v"


def build_program(rpc=RPC, c=C, n_cores=N_CORES):
    import concourse.bass as bass
    import concourse.mybir as mybir
    import concourse.tile as tile
    from concourse import bacc
    from concourse.alu_op_type import AluOpType as OP

    f32 = mybir.dt.float32
    f16 = mybir.dt.float16
    bf16 = mybir.dt.bfloat16
    u32 = mybir.dt.uint32
    ACT = mybir.ActivationFunctionType
    AX = mybir.AxisListType

    nstrip = rpc // P
    inv_n = 1.0 / (c - 1)

    nc = bacc.Bacc(None, target_bir_lowering=False, debug=False,
                   num_devices=n_cores)

    z_s = nc.declare_dram_parameter("z_s", [rpc, c], f32, isOutput=False)
    z_t = nc.declare_dram_parameter("z_t", [rpc, c], f32, isOutput=False)
    out = nc.declare_dram_parameter("out", [1, 1], f32, isOutput=True)

    def bcast(ap, dims):
        return bass.AP(tensor=ap.tensor, offset=ap.offset,
                       ap=[ap.ap[0]] + dims)

    from contextlib import ExitStack
    with tile.TileContext(nc) as tc, ExitStack() as ctx:
        zpool = ctx.enter_context(tc.tile_pool(name="zpool", bufs=2))
        upool = ctx.enter_context(tc.tile_pool(name="upool", bufs=2))
        scrp = ctx.enter_context(tc.tile_pool(name="scrp", bufs=1))
        sqpool = ctx.enter_context(tc.tile_pool(name="sqpool", bufs=2))
        small = ctx.enter_context(tc.tile_pool(name="small", bufs=1))
        stiny = ctx.enter_context(tc.tile_pool(name="stiny", bufs=2))
        psum = ctx.enter_context(tc.tile_pool(name="psum", bufs=2,
                                              space="PSUM"))
        dram = ctx.enter_context(tc.tile_pool(name="dram", bufs=1,
                                              space="DRAM"))

        # ---------------- persistent small tiles ----------------
        cnt = small.tile([P, nstrip, 2, RANK_CLAMP, 4], f32, tag="cnt")
        nc.vector.memset(cnt[:], 0.0)
        ssum = small.tile([P, nstrip, 2, NZB], f32, tag="ssum")
        thetas = small.tile([P, nstrip, 2, RANK_CLAMP], f32, tag="thetas")
        bt = small.tile([P, nstrip, 2, 2], f32, tag="bt")  # [b*, t*]
        stats_acc = small.tile([MP, NSTATS, NCH], f32, tag="stats_acc")
        nc.vector.memset(stats_acc[:], 0.0)
        stats_acc2 = small.tile([MP, NSTATS, NCH], f32, tag="stats_acc2")
        nc.vector.memset(stats_acc2[:], 0.0)
        nbias = small.tile([P, 1], f32, tag="nbias")
        nc.vector.memset(nbias[:], -EXP_BIAS)
        zbias = small.tile([P, 1], f32, tag="zbias")
        nc.vector.memset(zbias[:], 0.0)
        ones_col = small.tile([P, 1], f32, tag="ones_col")
        nc.vector.memset(ones_col[:], 1.0)

        cc_in = dram.tile([P, NSTATS * NCH], bf16, tag="cc_in")
        cc_out = dram.tile([16, NSTATS * NCH], bf16, tag="cc_out")
        cc_in2 = dram.tile([P, NSTATS * NCH], bf16, tag="cc_in2")
        cc_out2 = dram.tile([16, NSTATS * NCH], bf16, tag="cc_out2")
        ar_in = dram.tile([1, 4], f32, tag="ar_in")
        ar_out = dram.tile([1, 4], f32, tag="ar_out")

        # zero pad rows 125..127 of cc_in once
        zpad = small.tile([3, NSTATS * NCH], bf16, tag="zpad")
        nc.vector.memset(zpad[:], 0.0)
        nc.sync.dma_start(out=cc_in[MP:P, :], in_=zpad[:])
        nc.sync.dma_start(out=cc_in2[MP:P, :], in_=zpad[:])

        # scratch (single-buffered; consumers are same-engine in-order)
        scr_a = scrp.tile([P, HB], f16, tag="scr_a")     # ACT sign output
        vscr = scrp.tile([P, HB], f16, tag="vscr")       # DVE bitmap + folds


        def rsqrt_dve(dst, srcf, scr_f, scr_f2, y0):
            """dst = 1/sqrt(srcf), DVE only (no ACT table switch).
            Newton from constant seed y0 with clamped factor: globally
            convergent from above for any positive srcf."""
            y = scr_f
            t1 = scr_f2
            nc.vector.memset(y, y0)
            for _ in range(6):
                nc.vector.tensor_tensor(t1, y, y, OP.mult)
                nc.vector.tensor_tensor(t1, t1, srcf, OP.mult)
                nc.vector.tensor_scalar(t1, t1, -0.5, 1.5,
                                        OP.mult, OP.add)
                nc.vector.tensor_scalar_max(t1, t1, 0.25)
                nc.vector.tensor_tensor(y, y, t1, OP.mult)
            nc.vector.tensor_copy(dst, y)

        # =================== main strip loop ===================
        for s in range(nstrip):
            us = []
            # ---- DMA + exp (per z-block), u f16 [P, 16000] per tensor
            for t, zp in ((0, z_s), (1, z_t)):
                uq = upool.tile([P, c], f16, tag=f"u{t}")
                us.append(uq)
                for h in range(NZB):
                    zb = zpool.tile([P, ZB], f32, tag="zb")
                    col0 = h * ZB
                    nc.sync.dma_start(
                        out=zb[:],
                        in_=zp[s * P:(s + 1) * P, col0:col0 + ZB])
                    nc.scalar.activation(
                        uq[:, col0:col0 + ZB], zb[:], ACT.Exp,
                        bias=nbias[:], scale=1.0,
                        accum_out=ssum[:, s, t, h:h + 1])

            # ---- thetas from cols 0..9 (on ACT: keeps the ACT stream
            # self-contained so strip s+1 counts never wait on DVE)
            for t in range(2):
                nc.scalar.activation(thetas[:, s, t, :], us[t][:, 0:10],
                                     ACT.Copy, bias=0.0, scale=1.0)

            # ---- weights from row sums
            sS = stiny.tile([P, 2], f32, tag="sS")
            nc.vector.reduce_sum(sS[:, 0:1], ssum[:, s, 0, :], axis=AX.X)
            nc.vector.reduce_sum(sS[:, 1:2], ssum[:, s, 1, :], axis=AX.X)
            rr = stiny.tile([P, 2], f32, tag="rr")
            nc.vector.reciprocal(rr[:], sS[:])
            wf = stiny.tile([P, NSTATS], f32, tag="wf")
            nc.vector.tensor_scalar(wf[:, 0:1], rr[:, 0:1], A_SC, None,
                                    OP.mult)
            nc.vector.tensor_scalar(wf[:, 2:3], rr[:, 1:2], A_SC, None,
                                    OP.mult)
            r2 = stiny.tile([P, 3], f32, tag="r2")
            nc.vector.tensor_tensor(r2[:, 0:1], rr[:, 0:1], rr[:, 0:1],
                                    OP.mult)
            nc.vector.tensor_tensor(r2[:, 1:2], rr[:, 1:2], rr[:, 1:2],
                                    OP.mult)
            nc.vector.tensor_tensor(r2[:, 2:3], rr[:, 0:1], rr[:, 1:2],
                                    OP.mult)
            nc.vector.tensor_scalar(wf[:, 1:2], r2[:, 0:1], B_SC, None,
                                    OP.mult)
            nc.vector.tensor_scalar(wf[:, 3:4], r2[:, 1:2], B_SC, None,
                                    OP.mult)
            nc.vector.tensor_scalar(wf[:, 4:5], r2[:, 2:3], B_SC, None,
                                    OP.mult)
            wcol = stiny.tile([P, NSTATS], f16, tag="wcol")
            nc.vector.tensor_copy(wcol[:], wf[:])

            # ---- A-stats (u-stationary) early: PE runs these while
            # counts are in flight, before the GPS squares convoy
            stats_ps = psum.tile([MP, NSTATS, NCH], f32, tag="stats_ps")
            for kk in range(NCH):
                ksl = slice(kk * MP, (kk + 1) * MP)
                nc.tensor.matmul(stats_ps[0:MP, 0, kk:kk + 1],
                                 us[0][:, ksl], wcol[:, 0:1],
                                 start=True, stop=True)
                nc.tensor.matmul(stats_ps[0:MP, 2, kk:kk + 1],
                                 us[1][:, ksl], wcol[:, 2:3],
                                 start=True, stop=True)

            # ---- counts (20 units)
            for cc_ in range(RANK_CLAMP):
                for t in range(2):
                    th = thetas[:, s, t, cc_:cc_ + 1]
                    uq = us[t]
                    if CMP_ENGINE[(t, cc_)] == "a":
                        for g in range(2):
                            nc.scalar.activation(
                                scr_a[:], uq[:, g * HB:(g + 1) * HB],
                                ACT.Sign, bias=th, scale=-1.0,
                                accum_out=cnt[:, s, t, cc_, g:g + 1])
                    else:
                        for hh in range(2):
                            nc.vector.tensor_scalar(
                                vscr[:], uq[:, hh * HB:(hh + 1) * HB],
                                th, None, OP.is_lt)
                            # fold-add tree: 8000 -> 500 (in-place, f16
                            # partial sums stay exact: max value 16 < 2048)
                            w = HB // 2
                            while w >= 500:
                                nc.vector.tensor_tensor(
                                    vscr[:, 0:w], vscr[:, 0:w],
                                    vscr[:, w:2 * w], OP.add)
                                w //= 2
                            nc.vector.reduce_sum(
                                cnt[:, s, t, cc_, hh:hh + 1],
                                vscr[:, 0:500], axis=AX.X)

            # ---- argmax tournaments per tensor (full width)
            for t in range(2):
                uq = us[t]
                # T-fold: contiguous halving 16000 -> 250 into vscr
                T_glob = stiny.tile([P, 250], f16, tag="T_glob")
                nc.vector.tensor_tensor(vscr[:], uq[:, 0:8000],
                                        uq[:, 8000:16000], OP.max)
                w = HB // 2
                while w >= 250:
                    nc.vector.tensor_tensor(vscr[:, 0:w], vscr[:, 0:w],
                                            vscr[:, w:2 * w], OP.max)
                    w //= 2
                nc.vector.tensor_copy(T_glob[:], vscr[:, 0:250])
                # N-fold: 64 blocks of 250 -> [P, 64]
                M_N = stiny.tile([P, 64], f16, tag="M_N")
                vN = uq[:].rearrange("p (b i) -> p b i", b=64)
                tN = vscr[:].rearrange("p (b i) -> p b i", b=64)
                nc.vector.tensor_tensor(tN[:, :, 0:125], vN[:, :, 0:125],
                                        vN[:, :, 125:250], OP.max)
                curn = 125
                while curn > 1:
                    half = curn // 2
                    rem = curn - half  # fold upper half onto lower
                    nc.vector.tensor_tensor(
                        tN[:, :, 0:half], tN[:, :, 0:half],
                        tN[:, :, rem:curn], OP.max)
                    curn = rem
                nc.vector.tensor_copy(M_N[:], tN[:, :, 0])
                # finalize: j = 250*argmax(M_N) + argmax(T_glob)
                m8 = stiny.tile([P, 8], f16, tag="m8")
                i8 = stiny.tile([P, 8], u32, tag="i8")
                nc.vector.max(m8[:], M_N[:])
                m8b = bcast(m8[:, 0:1], [[0, 8]])
                nc.vector.max_index(i8[:], m8b, M_N[:])
                nc.vector.tensor_copy(bt[:, s, t, 0:1], i8[:, 0:1])
                nc.vector.max_index(i8[:], m8b, T_glob[:])
                nc.vector.tensor_copy(bt[:, s, t, 1:2], i8[:, 0:1])

            # ---- squares / cross on GPSIMD, consumed by PE
            SQB = 1000
            for q in range(c // SQB):
                qsl = slice(q * SQB, (q + 1) * SQB)
                sq_s = sqpool.tile([P, SQB], bf16, tag="sq_s")
                sq_t = sqpool.tile([P, SQB], bf16, tag="sq_t")
                xst = sqpool.tile([P, SQB], bf16, tag="xst")
                nc.gpsimd.tensor_tensor(sq_s[:], us[0][:, qsl], us[0][:, qsl],
                                        OP.mult)
                nc.gpsimd.tensor_tensor(sq_t[:], us[1][:, qsl], us[1][:, qsl],
                                        OP.mult)
                nc.gpsimd.tensor_tensor(xst[:], us[0][:, qsl], us[1][:, qsl],
                                        OP.mult)
                for k in range(SQB // MP):  # 16 chunks / block
                    kk = q * (SQB // MP) + k
                    ksl = slice(q * SQB + k * MP, q * SQB + (k + 1) * MP)
                    ksl_l = slice(k * MP, (k + 1) * MP)
                    lhss = ((sq_s[:, ksl_l], 1), (sq_t[:, ksl_l], 3),
                            (xst[:, ksl_l], 4))
                    for lhsT, si in lhss:
                        nc.tensor.matmul(
                            stats_ps[0:MP, si, kk:kk + 1],
                            lhsT, wcol[:, si:si + 1],
                            start=True, stop=True)
            sacc = stats_acc if s < 2 else stats_acc2
            nc.vector.tensor_tensor(sacc[:], sacc[:],
                                    stats_ps[0:MP, :, :], OP.add)
            if s == 3:
                stats_bf2 = small.tile([MP, NSTATS * NCH], bf16,
                                       tag="stats_bf2")
                nc.vector.tensor_copy(
                    stats_bf2[:],
                    stats_acc2[:].rearrange("p a b -> p (a b)"))
                nc.sync.dma_start(out=cc_in2[0:MP, :], in_=stats_bf2[:])
                nc.gpsimd.collective_compute(
                    "ReduceScatter", OP.add,
                    replica_groups=[list(range(n_cores))],
                    ins=[cc_in2[:].opt()], outs=[cc_out2[:].opt()])
            if s == 1:
                # first-half stats: start the ReduceScatter now so it
                # overlaps strips 2-3
                stats_bf1 = small.tile([MP, NSTATS * NCH], bf16,
                                       tag="stats_bf1")
                nc.vector.tensor_copy(
                    stats_bf1[:], stats_acc[:].rearrange("p a b -> p (a b)"))
                nc.sync.dma_start(out=cc_in[0:MP, :], in_=stats_bf1[:])
                nc.gpsimd.collective_compute(
                    "ReduceScatter", OP.add,
                    replica_groups=[list(range(n_cores))],
                    ins=[cc_in[:].opt()], outs=[cc_out[:].opt()])

        # ================= inter-term combine =================
        # js/jt = 250*b + t
        js = small.tile([P, nstrip], f32, tag="js")
        jt = small.tile([P, nstrip], f32, tag="jt")
        for t, jx in ((0, js), (1, jt)):
            nc.vector.tensor_scalar(jx[:], bt[:, :, t, 0], 250.0, None,
                                    OP.mult)
            nc.vector.tensor_tensor(jx[:], jx[:], bt[:, :, t, 1], OP.add)

        # counts: reduce over halves
        cr = small.tile([P, nstrip, 2, RANK_CLAMP, 1], f32, tag="cr")
        nc.vector.reduce_sum(cr[:], cnt[:], axis=AX.X)
        # ACT units: sign-sum acc = 2*r - (c-1) over non-self elements ->
        # r = 0.5*acc + 0.5*(c-1)
        for t in range(2):
            for cc_ in range(RANK_CLAMP):
                if CMP_ENGINE[(t, cc_)] == "a":
                    v = cr[:, :, t, cc_, 0]
                    nc.vector.tensor_scalar(v, v, float(c - 1), 0.5,
                                            OP.add, OP.mult)

        crs2 = cr[:, :, 0, :, 0]  # [P, strip, 10]
        crt2 = cr[:, :, 1, :, 0]
        js_b = bcast(js[:], [[1, nstrip], [0, RANK_CLAMP]])
        jt_b = bcast(jt[:], [[1, nstrip], [0, RANK_CLAMP]])
        gt_s = small.tile([P, nstrip, 10], f32, tag="gt_s")
        gt_t = small.tile([P, nstrip, 10], f32, tag="gt_t")
        kp_s = small.tile([P, nstrip, 10], f32, tag="kp_s")
        kp_t = small.tile([P, nstrip, 10], f32, tag="kp_t")
        p_s = small.tile([P, nstrip, 10], f32, tag="p_s")
        p_t = small.tile([P, nstrip, 10], f32, tag="p_t")
        for crx, jb, gt, kp, px, sent in (
                (crs2, js_b, gt_s, kp_s, p_s, 5.0),
                (crt2, jt_b, gt_t, kp_t, p_t, 7.0)):
            nc.vector.tensor_tensor(gt[:], crx, jb, OP.is_gt)
            nc.vector.tensor_tensor(kp[:], crx, jb, OP.not_equal)
            nc.vector.tensor_tensor(px[:], crx, gt[:], OP.subtract)
            nc.vector.tensor_scalar_add(px[:], px[:], sent)
            nc.vector.tensor_tensor(px[:], px[:], kp[:], OP.mult)
            nc.vector.tensor_scalar_add(px[:], px[:], -sent)

        wa = small.tile([P, 10], f32, tag="wa")
        for cc_ in range(RANK_CLAMP):
            nc.vector.memset(wa[:, cc_:cc_ + 1], float(cc_ - RANK_CLAMP))
        wa_b = bcast(wa[:], [[0, nstrip], [1, 10]])
        kw_s = small.tile([P, nstrip, 10], f32, tag="kw_s")
        kw_t = small.tile([P, nstrip, 10], f32, tag="kw_t")
        nc.vector.tensor_tensor(kw_s[:], kp_s[:], wa_b, OP.mult)
        nc.vector.tensor_tensor(kw_t[:], kp_t[:], wa_b, OP.mult)
        s1_s = small.tile([P, nstrip, 1], f32, tag="s1_s")
        s1_t = small.tile([P, nstrip, 1], f32, tag="s1_t")
        nc.vector.reduce_sum(s1_s[:], kw_s[:], axis=AX.X)
        nc.vector.reduce_sum(s1_t[:], kw_t[:], axis=AX.X)
        k2_s = small.tile([P, nstrip, 10], f32, tag="k2_s")
        k2_t = small.tile([P, nstrip, 10], f32, tag="k2_t")
        nc.vector.tensor_tensor(k2_s[:], kw_s[:], wa_b, OP.mult)
        nc.vector.tensor_tensor(k2_t[:], kw_t[:], wa_b, OP.mult)
        s2_s = small.tile([P, nstrip, 1], f32, tag="s2_s")
        s2_t = small.tile([P, nstrip, 1], f32, tag="s2_t")
        nc.vector.reduce_sum(s2_s[:], k2_s[:], axis=AX.X)
        nc.vector.reduce_sum(s2_t[:], k2_t[:], axis=AX.X)

        w100 = small.tile([P, 100], f32, tag="w100")
        nc.vector.tensor_tensor(
            w100[:],
            bcast(wa[:], [[1, 10], [0, 10]]),
            bcast(wa[:], [[0, 10], [1, 10]]), OP.mult)
        eq = small.tile([P, nstrip, 10, 10], f32, tag="eq")
        nc.vector.tensor_tensor(
            eq[:],
            bcast(p_s[:], [[10, nstrip], [1, 10], [0, 10]]),
            bcast(p_t[:], [[10, nstrip], [0, 10], [1, 10]]), OP.is_equal)
        nc.vector.tensor_tensor(
            eq[:], eq[:],
            bcast(w100[:], [[0, nstrip], [10, 10], [1, 10]]), OP.mult)
        xterm = small.tile([P, nstrip, 1, 1], f32, tag="xterm")
        nc.vector.reduce_sum(xterm[:], eq[:], axis=AX.XY)

        x2 = xterm[:, :, 0, 0]
        num = small.tile([P, nstrip], f32, tag="num")
        nc.vector.tensor_tensor(num[:], s1_s[:, :, 0], s1_t[:, :, 0], OP.mult)
        nc.vector.scalar_tensor_tensor(
            num[:], num[:], -inv_n, x2, OP.mult, OP.add)
        var_s = small.tile([P, nstrip], f32, tag="var_s")
        var_t = small.tile([P, nstrip], f32, tag="var_t")
        for s1x, s2x, varx in ((s1_s, s2_s, var_s), (s1_t, s2_t, var_t)):
            nc.vector.tensor_tensor(varx[:], s1x[:, :, 0], s1x[:, :, 0],
                                    OP.mult)
            nc.vector.scalar_tensor_tensor(
                varx[:], varx[:], -inv_n, s2x[:, :, 0], OP.mult, OP.add)
        den = small.tile([P, nstrip], f32, tag="den")
        nc.vector.tensor_tensor(den[:], var_s[:], var_t[:], OP.mult)
        rsqrt_dve(var_s[:], den[:], var_t[:], gt_s[:, :, 0], 3.0e-3)
        rho = small.tile([P, nstrip], f32, tag="rho")
        nc.vector.tensor_tensor(rho[:], num[:], var_s[:], OP.mult)
        eqj = small.tile([P, nstrip], f32, tag="eqj")
        nc.vector.tensor_tensor(eqj[:], js[:], jt[:], OP.is_equal)

        packed = small.tile([P, 2], f32, tag="packed")
        nc.vector.reduce_sum(packed[:, 0:1], rho[:], axis=AX.X)
        nc.vector.reduce_sum(packed[:, 1:2], eqj[:], axis=AX.X)
        inter_ps = psum.tile([1, 2], f32, tag="inter_ps")
        nc.tensor.matmul(inter_ps[:], ones_col[:], packed[:],
                         start=True, stop=True)
        inter_sb = small.tile([1, 2], f32, tag="inter_sb")
        nc.vector.tensor_copy(inter_sb[:], inter_ps[:])

        # ================= stats collective (bf16) =================
        # (RS2 emitted at end of strip 3; RS1 after strip 1)
        # per-rank pearson shard: [16, 5, 128] = sum of both halves
        shb = small.tile([16, NSTATS, NCH], bf16, tag="shb")
        nc.sync.dma_start(out=shb[:].rearrange("p a b -> p (a b)"),
                          in_=cc_out[:])
        shb2 = small.tile([16, NSTATS, NCH], bf16, tag="shb2")
        nc.sync.dma_start(out=shb2[:].rearrange("p a b -> p (a b)"),
                          in_=cc_out2[:])
        sh = small.tile([16, NSTATS, NCH], f32, tag="sh")
        nc.vector.tensor_tensor(sh[:], shb[:], shb2[:], OP.add)
        a_s, b_s, a_t, b_t, e_st = (sh[:, i, :] for i in range(NSTATS))
        inv_b = 1.0 / (rpc * n_cores)
        num2 = small.tile([16, NCH], f32, tag="num2")
        nc.vector.tensor_tensor(num2[:], a_s, a_t, OP.mult)
        nc.vector.scalar_tensor_tensor(
            num2[:], num2[:], -inv_b, e_st, OP.mult, OP.add)
        va = small.tile([16, NCH], f32, tag="va")
        vb = small.tile([16, NCH], f32, tag="vb")
        for ax, bx, vx in ((a_s, b_s, va), (a_t, b_t, vb)):
            nc.vector.tensor_tensor(vx[:], ax, ax, OP.mult)
            nc.vector.scalar_tensor_tensor(
                vx[:], vx[:], -inv_b, bx, OP.mult, OP.add)
        den2 = small.tile([16, NCH], f32, tag="den2")
        nc.vector.tensor_tensor(den2[:], va[:], vb[:], OP.mult)
        rsqrt_dve(va[:], den2[:], vb[:], sh[:, 1, :], 0.55)
        nc.vector.tensor_tensor(num2[:], num2[:], va[:], OP.mult)
        rho_cls = small.tile([16, 1], f32, tag="rho_cls")
        nc.vector.reduce_sum(rho_cls[:], num2[:], axis=AX.X)
        intra_ps = psum.tile([1, 1], f32, tag="intra_ps")
        nc.tensor.matmul(intra_ps[:], ones_col[0:16, :], rho_cls[:],
                         start=True, stop=True)

        # tiny AllReduce: [intra_shard, rho_sum, eq_sum, 0]
        sc4 = small.tile([1, 4], f32, tag="sc4")
        nc.vector.memset(sc4[:], 0.0)
        nc.vector.tensor_copy(sc4[:, 0:1], intra_ps[:])
        nc.vector.tensor_copy(sc4[:, 1:3], inter_sb[:])
        nc.sync.dma_start(out=ar_in[:], in_=sc4[:])
        nc.gpsimd.collective_compute(
            "AllReduce", OP.add,
            replica_groups=[list(range(n_cores))],
            ins=[ar_in[:].opt()], outs=[ar_out[:].opt()])
        scf = small.tile([1, 4], f32, tag="scf")
        nc.sync.dma_start(out=scf[:], in_=ar_out[:])

        # fin = 2 - (rho_sum + eq_sum)/B - intra_sum/C
        fin = small.tile([1, 1], f32, tag="fin")
        nc.vector.tensor_tensor(fin[:], scf[:, 1:2], scf[:, 2:3], OP.add)
        nc.vector.tensor_scalar_mul(fin[:], fin[:], -inv_b)
        nc.vector.scalar_tensor_tensor(
            fin[:], scf[:, 0:1], -1.0 / c, fin[:], OP.mult, OP.add)
        nc.vector.tensor_scalar_add(fin[:], fin[:], 2.0)
        nc.sync.dma_start(out=out[:], in_=fin[:])

    nc.finalize()
    return nc


_CACHED = {}


def _get_program():
    if "nc" not in _CACHED:
        _CACHED["nc"] = build_program()
    return _CACHED["nc"]


def kernel(z_s: np.ndarray, z_t: np.ndarray) -> np.ndarray:
    from concourse.bass_utils import run_bass_kernel_spmd

    nc = _get_program()
    in_maps = []
    for i in range(N_CORES):
        sl = slice(i * RPC, (i + 1) * RPC)
        in_maps.append({
            "z_s": np.ascontiguousarray(z_s[sl], dtype=np.float32),
            "z_t": np.ascontiguousarray(z_t[sl], dtype=np.float32),
        })
    res = run_bass_kernel_spmd(nc, in_maps, core_ids=list(range(N_CORES)))
    val = np.asarray(res.results[0]["out"], dtype=np.float32).reshape(())
    return val
